# revision 1
# baseline (speedup 1.0000x reference)
"""Multi-head attention (B=2, S=2048, D=1024, H=16) on 8 TRN2 NeuronCores.

Tensor-parallel over heads: core c owns heads {2c, 2c+1} (a 128-wide slice of
the qkv projections / a 128-row slice of Wo). Each core computes its partial
out-projection; the host sums the 8 partials and adds the bias.

Per-core pipeline (all matmuls in float32r, ~1.6e-4 rel err, full PE rate):
  - QT/KT = (q @ Wq|k)^T in [c, s] layout (lhsT = W chunk, rhs = qT chunk)
  - VT likewise, then PE-transposed into vh [k, dv] tiles augmented with a
    ones column so the attn@V matmul also produces the softmax row-sums
  - scores computed transposed: sT[k, q] = KT_h.T-slice @ QT_h (contraction
    over head dim, both heads packed into the PE array via row tiling)
  - exp on ScalarE with fused 1/8 scale, no max subtraction (scores ~N(0,1))
  - AV: ctxT_u[dv+1, q] accumulated over 16 k-tiles
  - normalization: reciprocal of the rowsum row, K=1 ones-matmul broadcast
    across partitions, DVE multiply; head-1 lanes moved to partitions 64:128
    with 32-aligned cross-quadrant DVE copies
  - out projection: out[s, e] partial = ctxT_n.T-slice @ Wo_slice
"""

import numpy as np

import concourse.bass as bass
import concourse.mybir as mybir
import concourse.tile as tile
from concourse import bacc
from concourse.bass_utils import run_bass_kernel_spmd
from concourse.masks import make_identity

F32 = mybir.dt.float32
F32R = mybir.dt.float32r
EXP = mybir.ActivationFunctionType.Exp

B, S, D = 2, 2048, 1024
H, DH = 16, 64
NCORES = 8
C = (H // NCORES) * DH  # per-core ctx width = 128
BS = B * S  # 4096

_CACHED_NC = None


def _build():
    nc = bacc.Bacc("TRN2", target_bir_lowering=False, debug=False)

    qT_d = nc.dram_tensor("qT", [D, BS], F32R, kind="ExternalInput")
    # weights come host-prearranged as [128, 8*C]: partition-major chunks so
    # each SBUF partition loads one contiguous 4KB line
    wq_d = nc.dram_tensor("wq", [128, 8 * C], F32R, kind="ExternalInput")
    wk_d = nc.dram_tensor("wk", [128, 8 * C], F32R, kind="ExternalInput")
    wv_d = nc.dram_tensor("wv", [128, 8 * C], F32R, kind="ExternalInput")
    wo_d = nc.dram_tensor("wo", [C, D], F32R, kind="ExternalInput")
    out_d = nc.dram_tensor("out", [BS, D], F32, kind="ExternalOutput")

    with tile.TileContext(nc) as tc:
        with (
            tc.tile_pool(name="cp", bufs=1) as cp,
            tc.tile_pool(name="pp", bufs=1) as pp,
            tc.tile_pool(name="sp", bufs=4) as sp,
            tc.tile_pool(name="wp", bufs=2) as wp,
            tc.tile_pool(name="ps", bufs=2, space="PSUM") as ps,
        ):
            ident = cp.tile([128, 128], F32, tag="ident")
            make_identity(nc, ident[:])
            ones32 = cp.tile([65, 64], F32, tag="ones32")
            nc.vector.memset(ones32[:], 1.0)
            ones_bc = cp.tile([65, 64], F32R, tag="ones_bc")
            nc.vector.tensor_copy(ones_bc[:], ones32[:])
            onesv = cp.tile([128, 16], F32, tag="onesv")
            nc.vector.memset(onesv[:], 1.0)

            def load_qt(b, sbi, qt_t=None, chunks=range(8)):
                s0 = sbi * 512
                if qt_t is None:
                    qt_t = sp.tile(
                        [128, 8, 512], F32R, tag="qt", bufs=2, name=f"qt_{b}_{sbi}"
                    )
                for kc in chunks:
                    nc.sync.dma_start(
                        qt_t[:, kc, :],
                        qT_d[
                            kc * 128 : (kc + 1) * 128, b * S + s0 : b * S + s0 + 512
                        ],
                    )
                return qt_t

            # DMA order at start is critical (queues drain roughly in issue
            # order): first qT chunk, then wq so the first projection matmul
            # can start ~2us in, then the rest.
            qt_first = load_qt(0, 0, chunks=range(1))
            wq_sb = cp.tile([128, 8, C], F32R, tag="wq")
            wk_sb = cp.tile([128, 8, C], F32R, tag="wk")
            wv_sb = cp.tile([128, 8, C], F32R, tag="wv")
            nc.sync.dma_start(wq_sb[:, 0, :], wq_d[:, 0:C])
            nc.sync.dma_start(
                wq_sb[:, 1:8, :],
                wq_d[:, C : 8 * C].rearrange("p (o c) -> p o c", o=7),
            )
            load_qt(0, 0, qt_t=qt_first, chunks=range(1, 8))
            nc.sync.dma_start(wk_sb[:], wk_d.ap().rearrange("p (o c) -> p o c", o=8))
            nc.sync.dma_start(wv_sb[:], wv_d.ap().rearrange("p (o c) -> p o c", o=8))
            wo_sb = cp.tile([128, D], F32R, tag="wo")
            nc.sync.dma_start(wo_sb[:], wo_d[:, :])

            QT = [pp.tile([128, S], F32R, tag=f"QT{b}", name=f"QT{b}") for b in range(B)]
            KT = [pp.tile([128, S], F32R, tag=f"KT{b}", name=f"KT{b}") for b in range(B)]
            vh = [
                pp.tile([128, 16, 130], F32R, tag=f"vh{b}", name=f"vh{b}")
                for b in range(B)
            ]
            strip = [
                [
                    pp.tile(
                        [128, 16, 256], F32R, tag=f"strip{h}{p}", name=f"strip{h}{p}"
                    )
                    for p in range(2)
                ]
                for h in range(2)
            ]

            def proj(b, sbi, qt_t=None):
                """Project one 512-wide s-block of batch b into QT/KT and vh."""
                s0 = sbi * 512
                if qt_t is None:
                    qt_t = load_qt(b, sbi)
                for w_sb, dst in ((wq_sb, QT[b]), (wk_sb, KT[b])):
                    pt = ps.tile([128, 512], F32, tag="bcop", name=f"pj_{b}_{sbi}")
                    for kc in range(8):
                        nc.tensor.matmul(
                            pt[:],
                            w_sb[:, kc, :],
                            qt_t[:, kc, :],
                            start=(kc == 0),
                            stop=(kc == 7),
                        )
                    nc.vector.tensor_copy(dst[:, s0 : s0 + 512], pt[:])
                # V: project, then PE-transpose 128x128 tiles into vh
                pt = ps.tile([128, 512], F32, tag="bcop", name=f"pjv_{b}_{sbi}")
                for kc in range(8):
                    nc.tensor.matmul(
                        pt[:],
                        wv_sb[:, kc, :],
                        qt_t[:, kc, :],
                        start=(kc == 0),
                        stop=(kc == 7),
                    )
                vt_blk = sp.tile([128, 512], F32, tag="vt", bufs=2, name=f"vt_{b}_{sbi}")
                nc.vector.tensor_copy(vt_blk[:], pt[:])
                for t in range(4):
                    st = sbi * 4 + t
                    ptr = ps.tile([128, 128], F32, tag="bcop", name=f"vtr_{b}_{st}")
                    nc.tensor.transpose(
                        ptr[:], vt_blk[:, t * 128 : (t + 1) * 128], ident[:]
                    )
                    nc.vector.tensor_copy(
                        vh[b][:, st, 0:130].rearrange("p (g j) -> p g j", g=2, j=65)[
                            :, :, 0:64
                        ],
                        ptr[:].rearrange("p (g j) -> p g j", g=2, j=64),
                    )

            def vh_ones(b):
                nc.vector.tensor_copy(vh[b][:, :, 64], onesv[:])
                nc.vector.tensor_copy(vh[b][:, :, 129], onesv[:])

            QW = 256  # q-chunk width of the scores->exp->AV pipeline

            def scores_exp(b, c, par):
                """Scores + exp for q-chunk c into strip[h][par]."""
                q0 = c * QW
                for g in range(4):
                    pscr = [
                        ps.tile(
                            [128, 1024], F32, tag="scores", name=f"sc_{b}_{c}_{h}_{g}"
                        )
                        for h in range(2)
                    ]
                    # adjacent h0/h1 pairs per k-tile: the two K=64 matmuls
                    # occupy disjoint PE row groups (0/64) and run
                    # concurrently on hardware via row tiling
                    for j in range(4):
                        kt = g * 4 + j
                        for h in range(2):
                            hp = h * 64
                            nc.tensor.matmul(
                                pscr[h][:, j * QW : (j + 1) * QW],
                                KT[b][hp : hp + 64, kt * 128 : (kt + 1) * 128],
                                QT[b][hp : hp + 64, q0 : q0 + QW],
                                start=True,
                                stop=True,
                            )
                    for h in range(2):
                        nc.scalar.activation(
                            strip[h][par][:, 4 * g : 4 * g + 4, :],
                            pscr[h][:].rearrange("p (g j) -> p g j", g=4, j=QW),
                            EXP,
                            scale=0.125,
                        )

            def av_out(b, c, par):
                """attn@V (+rowsum), normalize, partial out-proj for chunk c."""
                pctx = []
                rc = wp.tile([65, 2 * QW], F32R, tag="rcp", name=f"rc_{b}_{c}")
                for h in range(2):
                    pc = ps.tile([65, QW], F32, tag="ctx", name=f"cx_{b}_{c}_{h}")
                    for kt in range(16):
                        nc.tensor.matmul(
                            pc[:],
                            vh[b][:, kt, h * 65 : (h + 1) * 65],
                            strip[h][par][:, kt, :],
                            start=(kt == 0),
                            stop=(kt == 15),
                        )
                    with nc.allow_low_precision(reason="softmax denominator f32r"):
                        nc.vector.reciprocal(
                            rc[64:65, h * QW : (h + 1) * QW], pc[64:65, :]
                        )
                    pctx.append(pc)
                # one broadcast matmul covers both heads' reciprocal rows
                pball = ps.tile([64, 2 * QW], F32, tag="bcop", name=f"bc_{b}_{c}")
                nc.tensor.matmul(
                    pball[:], ones_bc[64:65, :], rc[64:65, :], start=True, stop=True
                )
                pbc = [pball[:, 0:QW], pball[:, QW : 2 * QW]]

                ctxn = wp.tile([128, QW], F32R, tag="ctxn", name=f"cn_{b}_{c}")
                cu0 = wp.tile([64, QW], F32, tag="cu", name=f"cu0_{b}_{c}")
                nc.vector.tensor_copy(cu0[:], pctx[0][0:64, :])
                nc.vector.tensor_mul(ctxn[0:64, :], cu0[:], pbc[0])
                cu1 = wp.tile([64, QW], F32, tag="cu", name=f"cu1_{b}_{c}")
                nc.vector.tensor_copy(cu1[:], pctx[1][0:64, :])
                tm1 = wp.tile([64, QW], F32R, tag="tm1", bufs=1, name=f"tm1_{b}_{c}")
                nc.vector.tensor_mul(tm1[:], cu1[:], pbc[1])
                nc.vector.tensor_copy(ctxn[64:96, :], tm1[0:32, :])
                nc.vector.tensor_copy(ctxn[96:128, :], tm1[32:64, :])

                for sc in range(QW // 128):
                    ob = wp.tile([128, D], F32, tag="ob", name=f"ob_{b}_{c}_{sc}")
                    for eh in range(2):
                        po = ps.tile(
                            [128, 512], F32, tag="bcop", name=f"po_{b}_{c}_{sc}_{eh}"
                        )
                        nc.tensor.matmul(
                            po[:],
                            ctxn[:, sc * 128 : (sc + 1) * 128],
                            wo_sb[:, eh * 512 : (eh + 1) * 512],
                            start=True,
                            stop=True,
                        )
                        nc.vector.tensor_copy(ob[:, eh * 512 : (eh + 1) * 512], po[:])
                    r0 = b * S + c * QW + sc * 128
                    nc.sync.dma_start(out_d[r0 : r0 + 128, :], ob[:])

            vh_ones(0)
            vh_ones(1)
            proj(0, 0, qt_first)
            for sbi in range(1, 4):
                proj(0, sbi)
            NCH = S // QW  # chunks per batch
            for gc in range(2 * NCH):
                b, c = divmod(gc, NCH)
                scores_exp(b, c, gc % 2)
                if gc > 0:
                    pb_, pc_ = divmod(gc - 1, NCH)
                    av_out(pb_, pc_, (gc - 1) % 2)
                if gc % 2 == 1 and gc < NCH:
                    proj(1, gc // 2)
            av_out(1, NCH - 1, (2 * NCH - 1) % 2)

    nc.compile()
    return nc


def _get_nc():
    global _CACHED_NC
    if _CACHED_NC is None:
        _CACHED_NC = _build()
    return _CACHED_NC


def _in_maps(q, Wq, Wk, Wv, Wo):
    qT = np.ascontiguousarray(np.asarray(q, np.float32).reshape(BS, D).T)
    Wq = np.asarray(Wq, np.float32)
    Wk = np.asarray(Wk, np.float32)
    Wv = np.asarray(Wv, np.float32)
    Wo = np.asarray(Wo, np.float32)
    def warr(W, sl):
        # [D, C] slice -> [128, 8*C]: partition p holds chunks (o*128+p, :)
        w = W[:, sl].reshape(8, 128, C).transpose(1, 0, 2)
        return np.ascontiguousarray(w.reshape(128, 8 * C))

    maps = []
    for c in range(NCORES):
        sl = slice(c * C, (c + 1) * C)
        maps.append(
            {
                "qT": qT,
                "wq": warr(Wq, sl),
                "wk": warr(Wk, sl),
                "wv": warr(Wv, sl),
                "wo": np.ascontiguousarray(Wo[sl, :]),
            }
        )
    return maps


def run(q, Wq, Wk, Wv, Wo, bo, trace=False):
    nc = _get_nc()
    res = run_bass_kernel_spmd(
        nc, _in_maps(q, Wq, Wk, Wv, Wo), list(range(NCORES)), trace=trace
    )
    acc = np.zeros((BS, D), np.float64)
    for r in res.results:
        acc += r["out"]
    out = (acc + np.asarray(bo, np.float32).astype(np.float64)).astype(np.float32)
    return out.reshape(B, S, D), res


def kernel(q, Wq, Wk, Wv, Wo, bo):
    out, _ = run(q, Wq, Wk, Wv, Wo, bo)
    return out



# revision 6
# speedup vs baseline: 1.0395x; 1.0395x over previous
"""Multi-head attention (B=2, S=2048, D=1024, H=16) on 8 TRN2 NeuronCores.

Tensor-parallel over heads: core c owns heads {2c, 2c+1} (a 128-wide slice of
the QKV projections / a 128-row slice of Wo). Each core computes its partial
out-projection in fp16; the host sums the 8 partials and adds the bias.

v2 layout (all-bf16/fp16 matmuls, q-major AV):
  - QT/KT = (q @ Wq|k)^T in [dh2h, bs] layout (lhsT = W chunk, rhs = qT chunk)
  - V projected directly into vh [s(=k), dv2h] tiles (lhsT = qT chunk)
  - scores k-major: sT[k, q] = KT_h-slice.T @ QT_h-slice, psum [128, 1024]
  - exp on ScalarE (fused 1/8 scale, no max subtraction; scores ~N(0,1)),
    strip bf16 in SBUF
  - AV q-major: ctx[q, dv] += strip_slice.T @ vh_slice accumulated over the
    16 k-tiles; row-sums via an extra ones-column matmul per q-tile (out
    free size 1 -> ~free)
  - normalization fused into the ctx transpose: a regular matmul against
    diag(1/D) (built by GPSIMD from an fp16 identity) yields
    ctxT[dvh, q] = ctx[q, dvh]/D_q
  - out projection: po[q, e] = ctxT.T-slice @ Wo-slice, fp16 partial out

Scheduling: a global 128-slot pipeline (slot = one (pass, k-tile)); each slot
carries scores+exp+AV(lag 3) plus "filler" PE work (projections, V, out-proj)
budgeted so the tensor engine never idles (the cost model's p-state ramp
penalizes every PE idle gap). Warmup dummy matmuls cover the initial DMA wait.
"""

import numpy as np
import ml_dtypes

import concourse.bass as bass
import concourse.mybir as mybir
import concourse.tile as tile
from concourse import bacc
from concourse.bass_utils import run_bass_kernel_spmd
from concourse.masks import make_identity

BF16 = mybir.dt.bfloat16
FP16 = mybir.dt.float16
F32 = mybir.dt.float32
EXP = mybir.ActivationFunctionType.Exp

B, S, D = 2, 2048, 1024
H, DH = 16, 64
NCORES = 8
BS = B * S  # 4096
NCH = D // 128  # 8 contraction chunks for the projections
NKT = S // 128  # 16 k-tiles per batch
NQT = 8  # q-tiles (128) per q-half
AV_LAG = 3  # slots between exp(s) and AV(s)
N_DUMMY = 18  # warmup matmuls riding out the DMA wait + p-state ramp

# pass p = (b, qh, h); slot s = p*16 + kt
PASSES = [(b, qh, h) for b in range(B) for qh in range(2) for h in range(2)]
NSLOT = len(PASSES) * NKT  # 128

_CACHED_NC = None


def _build():
    nc = bacc.Bacc("TRN2", target_bir_lowering=False, debug=False)

    qt_d = nc.dram_tensor("qt", [128, NCH, BS], BF16, kind="ExternalInput")
    wq_d = nc.dram_tensor("wq", [128, NCH, 128], BF16, kind="ExternalInput")
    wk_d = nc.dram_tensor("wk", [128, NCH, 128], BF16, kind="ExternalInput")
    wv_d = nc.dram_tensor("wv", [128, NCH, 128], BF16, kind="ExternalInput")
    wo_d = nc.dram_tensor("wo", [128, D], FP16, kind="ExternalInput")
    out_d = nc.dram_tensor("out", [BS, D], FP16, kind="ExternalOutput")

    with tile.TileContext(nc) as tc:
        with (
            tc.tile_pool(name="cp", bufs=1) as cp,
            tc.tile_pool(name="sp", bufs=1) as sp,
            tc.tile_pool(name="ps", bufs=1, space="PSUM") as ps,
        ):
            # ---- persistent SBUF ----
            qt_sb = cp.tile([128, NCH, BS], BF16, tag="qt")
            wq_sb = cp.tile([128, NCH, 128], BF16, tag="wq")
            wk_sb = cp.tile([128, NCH, 128], BF16, tag="wk")
            wv_sb = cp.tile([128, NCH, 128], BF16, tag="wv")
            wo_sb = cp.tile([128, D], FP16, tag="wo")
            QTt = cp.tile([128, BS], BF16, tag="QT")  # [2h*dh, b*s]
            KTt = cp.tile([128, BS], BF16, tag="KT")
            vht = cp.tile([128, B, NKT, 128], BF16, tag="vh")  # [k, b, kt, dv2h]
            onesc = cp.tile([128, 1], BF16, tag="ones")
            identf = cp.tile([128, 128], FP16, tag="ident")
            dmy_w = cp.tile([128, 128], BF16, tag="dmy_w")
            dmy_a = cp.tile([128, 512], BF16, tag="dmy_a")

            # one shared PSUM bank for the small tiles (bank-granular
            # allocator): per-kt D partials [*,0:128] as [8qt,16kt], vv
            # [*,128:256], ptr ping-pong [*,256:384] / [*,384:512]
            smallp = ps.tile([128, 512], F32, tag="small", bufs=1)

            nc.vector.memset(onesc[:], 1.0)
            nc.vector.memset(dmy_w[:], 0.0)
            nc.vector.memset(dmy_a[:], 0.0)
            make_identity(nc, identf[:])

            # ---- warmup dummies (PE busy during DMA wait; ride the ramp) ----
            for i in range(N_DUMMY):
                pd = ps.tile(
                    [128, 1024], F32, tag="pscr", bufs=2, name=f"dmy{i}"
                )
                nc.tensor.matmul(
                    pd[:, 0:512], dmy_w[:], dmy_a[:], start=True, stop=True
                )

            # ---- input DMAs (queue order matters: first-needed first) ----
            nc.sync.dma_start(wq_sb[:], wq_d.ap())
            nc.sync.dma_start(wk_sb[:], wk_d.ap())
            for c in range(NCH):
                nc.sync.dma_start(qt_sb[:, c, 0:1024], qt_d[:, c, 0:1024])
            nc.sync.dma_start(wv_sb[:], wv_d.ap())
            for c in range(NCH):
                nc.sync.dma_start(qt_sb[:, c, 1024:BS], qt_d[:, c, 1024:BS])
            nc.sync.dma_start(wo_sb[:], wo_d[:, :])

            # ---- helpers ----
            proj_state = {}

            def proj_quarter(which, blk, quarter):
                """2 of the 8 contraction-chunk matmuls of one 512-col
                projection block; quarter 3 adds the PSUM->SBUF copy."""
                w_sb, dst = (wq_sb, QTt) if which == "Q" else (wk_sb, KTt)
                s0 = blk * 512
                key = (which, blk)
                if quarter == 0:
                    proj_state[key] = ps.tile(
                        [128, 512], F32, tag="pt", bufs=1, name=f"pt_{which}{blk}"
                    )
                pt = proj_state[key]
                for c in (2 * quarter, 2 * quarter + 1):
                    nc.tensor.matmul(
                        pt[:],
                        w_sb[:, c, :],
                        qt_sb[:, c, s0 : s0 + 512],
                        start=(c == 0),
                        stop=(c == 7),
                    )
                if quarter == 3:
                    nc.vector.tensor_copy(dst[:, s0 : s0 + 512], pt[:])

            def v_tile(bb, st):
                """Project one 128-row s-tile of batch bb into vh."""
                vv = smallp[:, 128:256]
                s0 = bb * S + st * 128
                for c in range(NCH):
                    nc.tensor.matmul(
                        vv[:],
                        qt_sb[:, c, s0 : s0 + 128],
                        wv_sb[:, c, :],
                        start=(c == 0),
                        stop=(c == 7),
                    )
                nc.vector.tensor_copy(vht[:, bb, st, :], vv[:])

            cc_tiles = {}
            rcp_tiles = {}
            ptr_idx = [0]

            def scores_exp(b, qh, h, kt, s):
                hp = h * 64
                k0 = b * S + kt * 128
                q0 = b * S + qh * 1024
                pscr = ps.tile([128, 1024], F32, tag="pscr", bufs=2, name=f"sc{s}")
                for j in range(2):
                    nc.tensor.matmul(
                        pscr[:, j * 512 : (j + 1) * 512],
                        KTt[hp : hp + 64, k0 : k0 + 128],
                        QTt[hp : hp + 64, q0 + j * 512 : q0 + (j + 1) * 512],
                        start=True,
                        stop=True,
                    )
                strip = sp.tile(
                    [128, 1024], BF16, tag="strip", bufs=26, name=f"st{s}"
                )
                nc.scalar.activation(strip[:], pscr[:], EXP, scale=0.125)
                return strip

            av_state = {}
            strips = {}

            def av_qt(p, qtt):
                """Full 16-kt accumulation for one q-tile of pass p (runs as
                a filler during pass p+1). Groups are contiguous: interleaved
                PSUM accumulation groups in one bank break (bank-level
                has_written clear on start)."""
                b, qh, h = PASSES[p]
                if qtt == 0:
                    av_state[p] = ps.tile(
                        [128, NQT, 64], F32, tag="avp", bufs=1, name=f"av{p}"
                    )
                avp = av_state[p]
                for kt in range(NKT):
                    nc.tensor.matmul(
                        avp[:, qtt, :],
                        strips[p * 16 + kt][:, qtt * 128 : (qtt + 1) * 128],
                        vht[:, b, kt, h * 64 : h * 64 + 64],
                        start=(kt == 0),
                        stop=(kt == NKT - 1),
                    )
                for kt in range(NKT):
                    nc.tensor.matmul(
                        smallp[:, qtt : qtt + 1],
                        strips[p * 16 + kt][:, qtt * 128 : (qtt + 1) * 128],
                        onesc[:],
                        start=(kt == 0),
                        stop=(kt == NKT - 1),
                    )
                if qtt == NQT - 1:
                    for kt in range(NKT):
                        strips.pop(p * 16 + kt)
                    drain(p)

            def drain(p):
                """recip of the row-sums + evacuate ctx into cc (SBUF fp16)."""
                b, qh, h = PASSES[p]
                avp = av_state.pop(p)
                if h == 0:
                    cc_tiles[(b, qh)] = cp.tile(
                        [128, NQT, 128], FP16, tag="cc", bufs=2, name=f"cc{b}{qh}"
                    )
                    rcp_tiles[(b, qh)] = cp.tile(
                        [128, 2, NQT], F32, tag="rcp", bufs=2, name=f"rcp{b}{qh}"
                    )
                cc = cc_tiles[(b, qh)]
                rcp = rcp_tiles[(b, qh)]
                with nc.allow_low_precision(reason="softmax denominator"):
                    nc.vector.reciprocal(rcp[:, h, :], smallp[:, 0:8])
                nc.vector.tensor_copy(cc[:, :, h * 64 : h * 64 + 64], avp[:])

            def outproj_a(b, qh, qtt):
                """diag(1/D) builds + transpose-normalize matmuls + ctxT copy."""
                cc = cc_tiles[(b, qh)]
                rcp = rcp_tiles[(b, qh)]
                pp = ptr_idx[0] % 2
                ptr_idx[0] += 1
                ptr = smallp[:, 256 + pp * 128 : 384 + pp * 128]
                for h in range(2):
                    dg = sp.tile(
                        [128, 128], FP16, tag="diag", bufs=3, name=f"dg{b}{qh}{qtt}{h}"
                    )
                    nc.gpsimd.tensor_scalar_mul(
                        dg[:], identf[:], rcp[:, h, qtt : qtt + 1]
                    )
                    nc.tensor.matmul(
                        ptr[h * 64 : (h + 1) * 64, :],
                        cc[:, qtt, h * 64 : (h + 1) * 64],
                        dg[:],
                        start=True,
                        stop=True,
                    )
                ctxT = sp.tile(
                    [128, 128], FP16, tag="ctxT", bufs=2, name=f"cx{b}{qh}{qtt}"
                )
                nc.vector.tensor_copy(ctxT[:], ptr[:])
                return ctxT

            def outproj_b(b, qh, qtt, ctxT, eh, tail=False):
                po = ps.tile(
                    [128, 512], F32, tag="po", bufs=1, name=f"po{b}{qh}{qtt}{eh}"
                )
                nc.tensor.matmul(
                    po[:],
                    ctxT[:],
                    wo_sb[:, eh * 512 : (eh + 1) * 512],
                    start=True,
                    stop=True,
                )
                ob = sp.tile(
                    [128, 512], FP16, tag="ob", bufs=3, name=f"ob{b}{qh}{qtt}{eh}"
                )
                if tail and eh == 1:
                    nc.scalar.copy(ob[:], po[:])  # ACT is free in the tail
                else:
                    nc.vector.tensor_copy(ob[:], po[:])
                r0 = b * S + qh * 1024 + qtt * 128
                nc.sync.dma_start(
                    out_d[r0 : r0 + 128, eh * 512 : (eh + 1) * 512], ob[:]
                )

            # ---- filler schedule ----
            # Each filler is (ready_slot, deadline_slot, fn). Deadlines are
            # asserted; ready respects data deps (proj blocks are sequential
            # through the single pt psum tile by construction of the list).
            fillers = []

            def F(ready, deadline, fn):
                fillers.append([ready, deadline, fn])

            # KT b0 remaining blocks (blk0 in prologue): scores kt needs KT
            # block kt//4 at slot kt.
            for blk, dl in ((1, 4), (2, 8), (3, 12)):
                for qq in range(4):
                    F(0, dl - 1, lambda blk=blk, qq=qq: proj_quarter("K", blk, qq))
            # V b0: vh[st] needed by AV kt=st at slot st+AV_LAG (st0,1 in prologue)
            for st in range(2, 16):
                F(0, st + AV_LAG - 1, lambda st=st: v_tile(0, st))
            # QT b0 blk2,3 (qh1 -> slot 32)
            for blk in (2, 3):
                for qq in range(4):
                    F(4, 31, lambda blk=blk, qq=qq: proj_quarter("Q", blk, qq))
            # b1 projections (scores from slot 64; KT block (kt//4)+4 at slot
            # 64+kt; QT blk4,5 by 64, blk6,7 by 96)
            for blk, dl in ((4, 63), (5, 67), (6, 71), (7, 75)):
                for qq in range(4):
                    F(16, dl, lambda blk=blk, qq=qq: proj_quarter("K", blk, qq))
            for blk, dl in ((4, 63), (5, 63), (6, 94), (7, 94)):
                for qq in range(4):
                    F(
                        18 if blk < 6 else 76,
                        dl,
                        lambda blk=blk, qq=qq: proj_quarter("Q", blk, qq),
                    )
            # V b1: needed from slot 64+st+AV_LAG
            for st in range(16):
                F(44, 64 + st + AV_LAG - 1, lambda st=st: v_tile(1, st))

            # out-projections become ready once both h-passes of (b, qh) have
            # drained: pass p=(b,qh,1) drains at slot p*16+15+AV_LAG.
            ctxT_holder = {}

            def op_a(b, qh, qtt):
                ctxT_holder[(b, qh, qtt)] = outproj_a(b, qh, qtt)

            def op_b(b, qh, qtt, eh):
                outproj_b(b, qh, qtt, ctxT_holder[(b, qh, qtt)], eh)

            # deferred AV: pass p's per-q-tile accumulations run during
            # pass p+1 (they need all 16 strips of pass p)
            for p in range(len(PASSES) - 1):
                for qtt in range(NQT):
                    F(
                        (p + 1) * 16 + qtt,
                        (p + 1) * 16 + qtt + 6,
                        lambda p=p, qtt=qtt: av_qt(p, qtt),
                    )

            for gi, (b, qh) in enumerate([(0, 0), (0, 1), (1, 0)]):
                rdy = (PASSES.index((b, qh, 1)) + 1) * 16 + 9
                for qtt in range(NQT):
                    F(rdy + 2 * qtt, NSLOT - 1, lambda b=b, qh=qh, qtt=qtt: op_a(b, qh, qtt))
                    F(rdy + 2 * qtt, NSLOT - 1, lambda b=b, qh=qh, qtt=qtt: op_b(b, qh, qtt, 0))
                    F(rdy + 2 * qtt + 1, NSLOT - 1, lambda b=b, qh=qh, qtt=qtt: op_b(b, qh, qtt, 1))

            fillers.sort(key=lambda f: (f[0], f[1]))

            # ---- prologue projections (needed before slot 0) ----
            for qq in range(4):
                proj_quarter("Q", 0, qq)
            for qq in range(4):
                proj_quarter("Q", 1, qq)
            for qq in range(4):
                proj_quarter("K", 0, qq)
            v_tile(0, 0)
            v_tile(0, 1)

            # ---- main pipeline ----
            done = [False] * len(fillers)

            def pop_fillers(s, budget):
                """Emit due/ready fillers for slot s up to a PE-cost budget."""
                spent = 0
                # urgent first (deadline within 2 slots), then earliest-ready
                for idx, (rdy, dl, fn) in enumerate(fillers):
                    if done[idx] or rdy > s:
                        continue
                    urgent = dl <= s + 2
                    if not urgent and spent >= budget:
                        continue
                    fn()
                    done[idx] = True
                    spent += 1
                    if spent >= budget and not any(
                        (not done[j]) and fillers[j][0] <= s and fillers[j][1] <= s + 2
                        for j in range(len(fillers))
                    ):
                        break
                return spent

            for s in range(NSLOT):
                p, kt = divmod(s, NKT)
                b, qh, h = PASSES[p]
                strips[s] = scores_exp(b, qh, h, kt, s)
                pop_fillers(s, budget=1)

            # ---- epilogue ----
            for idx, (rdy, dl, fn) in enumerate(fillers):
                if not done[idx]:
                    fn()
                    done[idx] = True
            for qtt in range(NQT):
                av_qt(len(PASSES) - 1, qtt)
            for qtt in range(NQT):
                ctxT = outproj_a(1, 1, qtt)
                for eh in range(2):
                    outproj_b(1, 1, qtt, ctxT, eh, tail=True)

    nc.compile()
    return nc


def _get_nc():
    global _CACHED_NC
    if _CACHED_NC is None:
        _CACHED_NC = _build()
    return _CACHED_NC


def _in_maps(q, Wq, Wk, Wv, Wo):
    q32 = np.asarray(q, np.float32).reshape(BS, NCH, 128)
    qt = np.ascontiguousarray(q32.transpose(2, 1, 0)).astype(ml_dtypes.bfloat16)

    def warr(W, sl):
        w = np.asarray(W, np.float32)[:, sl].reshape(NCH, 128, 128).transpose(1, 0, 2)
        return np.ascontiguousarray(w).astype(ml_dtypes.bfloat16)

    Wo32 = np.asarray(Wo, np.float32)
    maps = []
    for c in range(NCORES):
        sl = slice(c * 128, (c + 1) * 128)
        maps.append(
            {
                "qt": qt,
                "wq": warr(Wq, sl),
                "wk": warr(Wk, sl),
                "wv": warr(Wv, sl),
                "wo": np.ascontiguousarray(Wo32[sl, :]).astype(np.float16),
            }
        )
    return maps


def run(q, Wq, Wk, Wv, Wo, bo, trace=False):
    nc = _get_nc()
    res = run_bass_kernel_spmd(
        nc, _in_maps(q, Wq, Wk, Wv, Wo), list(range(NCORES)), trace=trace
    )
    acc = np.zeros((BS, D), np.float32)
    for r in res.results:
        acc += r["out"].astype(np.float32)
    out = (acc + np.asarray(bo, np.float32)).astype(np.float32)
    return out.reshape(B, S, D), res


def kernel(q, Wq, Wk, Wv, Wo, bo):
    out, _ = run(q, Wq, Wk, Wv, Wo, bo)
    return out


# revision 10
# speedup vs baseline: 1.0533x; 1.0132x over previous
"""Multi-head attention (B=2, S=2048, D=1024, H=16) on 8 TRN2 NeuronCores.

Tensor-parallel over heads: core c owns heads {2c, 2c+1} (a 128-wide slice of
the QKV projections / a 128-row slice of Wo). Each core computes its partial
out-projection in fp16; the host sums the 8 partials and adds the bias.

v2 layout (all-bf16/fp16 matmuls, q-major AV):
  - QT/KT = (q @ Wq|k)^T in [dh2h, bs] layout (lhsT = W chunk, rhs = qT chunk)
  - V projected directly into vh [s(=k), dv2h] tiles (lhsT = qT chunk)
  - scores k-major: sT[k, q] = KT_h-slice.T @ QT_h-slice, psum [128, 1024]
  - exp on ScalarE (fused 1/8 scale, no max subtraction; scores ~N(0,1)),
    strip bf16 in SBUF
  - AV q-major: ctx[q, dv] += strip_slice.T @ vh_slice accumulated over the
    16 k-tiles; row-sums via an extra ones-column matmul per q-tile (out
    free size 1 -> ~free)
  - normalization fused into the ctx transpose: a regular matmul against
    diag(1/D) (built by GPSIMD from an fp16 identity) yields
    ctxT[dvh, q] = ctx[q, dvh]/D_q
  - out projection: po[q, e] = ctxT.T-slice @ Wo-slice, fp16 partial out

Scheduling: a global 128-slot pipeline (slot = one (pass, k-tile)); each slot
carries scores+exp+AV(lag 3) plus "filler" PE work (projections, V, out-proj)
budgeted so the tensor engine never idles (the cost model's p-state ramp
penalizes every PE idle gap). Warmup dummy matmuls cover the initial DMA wait.
"""

import numpy as np
import ml_dtypes

import concourse.bass as bass
import concourse.mybir as mybir
import concourse.tile as tile
from concourse import bacc
from concourse.bass_utils import run_bass_kernel_spmd
from concourse.masks import make_identity

BF16 = mybir.dt.bfloat16
FP16 = mybir.dt.float16
F32 = mybir.dt.float32
EXP = mybir.ActivationFunctionType.Exp

B, S, D = 2, 2048, 1024
H, DH = 16, 64
NCORES = 8
BS = B * S  # 4096
NCH = D // 128  # 8 contraction chunks for the projections
NKT = S // 128  # 16 k-tiles per batch
NQT = 8  # q-tiles (128) per q-half
AV_LAG = 3  # slots between exp(s) and AV(s)
N_DUMMY = 12  # warmup matmuls riding out the DMA wait + p-state ramp

# pass p = (b, qh, h); slot s = p*16 + kt
PASSES = [(b, qh, h) for b in range(B) for qh in range(2) for h in range(2)]
NSLOT = len(PASSES) * NKT  # 128

_CACHED_NC = None


def _build():
    nc = bacc.Bacc("TRN2", target_bir_lowering=False, debug=False)

    qt_d = nc.dram_tensor("qt", [128, NCH, BS], BF16, kind="ExternalInput")
    wq_d = nc.dram_tensor("wq", [128, NCH, 128], BF16, kind="ExternalInput")
    wk_d = nc.dram_tensor("wk", [128, NCH, 128], BF16, kind="ExternalInput")
    wv_d = nc.dram_tensor("wv", [128, NCH, 128], BF16, kind="ExternalInput")
    wo_d = nc.dram_tensor("wo", [128, D], FP16, kind="ExternalInput")
    out_d = nc.dram_tensor("out", [BS, D], FP16, kind="ExternalOutput")

    with tile.TileContext(nc) as tc:
        with (
            tc.tile_pool(name="cp", bufs=1) as cp,
            tc.tile_pool(name="sp", bufs=1) as sp,
            tc.tile_pool(name="ps", bufs=1, space="PSUM") as ps,
        ):
            # ---- persistent SBUF ----
            qt_sb = cp.tile([128, NCH, BS], BF16, tag="qt")
            wq_sb = cp.tile([128, NCH, 128], BF16, tag="wq")
            wk_sb = cp.tile([128, NCH, 128], BF16, tag="wk")
            wv_sb = cp.tile([128, NCH, 128], BF16, tag="wv")
            wo_sb = cp.tile([128, D], FP16, tag="wo")
            QTt = cp.tile([128, BS], BF16, tag="QT")  # [2h*dh, b*s]
            KTt = cp.tile([128, BS], BF16, tag="KT")
            vht = cp.tile([128, B, NKT, 128], BF16, tag="vh")  # [k, b, kt, dv2h]
            onesc = cp.tile([128, 1], BF16, tag="ones")
            identf = cp.tile([128, 128], FP16, tag="ident")
            dmy_w = cp.tile([128, 128], BF16, tag="dmy_w")
            dmy_a = cp.tile([128, 512], BF16, tag="dmy_a")

            # one shared PSUM bank for the small tiles (bank-granular
            # allocator): per-kt D partials [*,0:128] as [8qt,16kt], vv
            # [*,128:256], ptr ping-pong [*,256:384] / [*,384:512]
            smallp = ps.tile([128, 512], F32, tag="small", bufs=1)

            nc.vector.memset(onesc[:], 1.0)
            nc.vector.memset(dmy_w[:], 0.0)
            nc.vector.memset(dmy_a[:], 0.0)
            make_identity(nc, identf[:])

            # ---- warmup dummies (PE busy during DMA wait; ride the ramp) ----
            for i in range(N_DUMMY):
                pd = ps.tile(
                    [128, 1024], F32, tag="pscr", bufs=2, name=f"dmy{i}"
                )
                nc.tensor.matmul(
                    pd[:, 0:512], dmy_w[:], dmy_a[:], start=True, stop=True
                )

            # ---- input DMAs: critical wave first, issue spread across
            # the SP/ACT/DVE DGE queues (issue itself serializes per engine)
            nc.sync.dma_start(wq_sb[:], wq_d.ap())
            nc.scalar.dma_start(wk_sb[:], wk_d.ap())
            nc.sync.dma_start(qt_sb[:, 0:4, 0:512], qt_d[:, 0:4, 0:512])
            nc.sync.dma_start(qt_sb[:, 4:8, 0:512], qt_d[:, 4:8, 0:512])
            nc.scalar.dma_start(qt_sb[:, 0:4, 512:1024], qt_d[:, 0:4, 512:1024])
            nc.scalar.dma_start(qt_sb[:, 4:8, 512:1024], qt_d[:, 4:8, 512:1024])
            nc.sync.dma_start(wv_sb[:], wv_d.ap())
            nc.sync.dma_start(qt_sb[:, 0:4, 1024:2560], qt_d[:, 0:4, 1024:2560])
            nc.scalar.dma_start(qt_sb[:, 4:8, 1024:2560], qt_d[:, 4:8, 1024:2560])
            nc.scalar.dma_start(qt_sb[:, 0:4, 2560:BS], qt_d[:, 0:4, 2560:BS])
            nc.sync.dma_start(qt_sb[:, 4:8, 2560:BS], qt_d[:, 4:8, 2560:BS])
            nc.scalar.dma_start(wo_sb[:], wo_d[:, :])

            # ---- helpers ----
            def proj_half(which, blk, half):
                """4 of the 8 contraction-chunk matmuls of one 512-col
                projection block. Each half owns its flex psum tile for its
                whole lifecycle (alloc->mms->evict) so the shared flex ring
                can never deadlock the PE FIFO. half 1 adds into the dst."""
                w_sb, dst = (wq_sb, QTt) if which == "Q" else (wk_sb, KTt)
                s0 = blk * 512
                pt = ps.tile(
                    [128, 512], F32, tag="flex", bufs=2, name=f"pt_{which}{blk}_{half}"
                )
                for c in range(half * 4, half * 4 + 4):
                    nc.tensor.matmul(
                        pt[:],
                        w_sb[:, c, :],
                        qt_sb[:, c, s0 : s0 + 512],
                        start=(c == half * 4),
                        stop=(c == half * 4 + 3),
                    )
                if half == 0:
                    nc.vector.tensor_copy(dst[:, s0 : s0 + 512], pt[:])
                else:
                    nc.vector.tensor_tensor(
                        dst[:, s0 : s0 + 512],
                        dst[:, s0 : s0 + 512],
                        pt[:],
                        mybir.AluOpType.add,
                    )

            def v_tile(bb, st):
                """Project one 128-row s-tile of batch bb into vh."""
                vv = smallp[:, 128:256]
                s0 = bb * S + st * 128
                for c in range(NCH):
                    nc.tensor.matmul(
                        vv[:],
                        qt_sb[:, c, s0 : s0 + 128],
                        wv_sb[:, c, :],
                        start=(c == 0),
                        stop=(c == 7),
                    )
                nc.vector.tensor_copy(vht[:, bb, st, :], vv[:])

            cc_tiles = {}
            rcp_tiles = {}
            ptr_idx = [0]

            def scores_exp(b, qh, h, kt, s):
                hp = h * 64
                k0 = b * S + kt * 128
                q0 = b * S + qh * 1024
                pscr = ps.tile([128, 1024], F32, tag="pscr", bufs=2, name=f"sc{s}")
                for j in range(2):
                    nc.tensor.matmul(
                        pscr[:, j * 512 : (j + 1) * 512],
                        KTt[hp : hp + 64, k0 : k0 + 128],
                        QTt[hp : hp + 64, q0 + j * 512 : q0 + (j + 1) * 512],
                        start=True,
                        stop=True,
                    )
                strip = sp.tile(
                    [128, 1024], BF16, tag="strip", bufs=32, name=f"st{s}"
                )
                nc.scalar.activation(strip[:], pscr[:], EXP, scale=0.125)
                return strip

            av_state = {}
            strips = {}

            def av_qt(p, qtt):
                """Full 16-kt accumulation for one q-tile of pass p (runs as
                a filler during pass p+1). Groups are contiguous: interleaved
                PSUM accumulation groups in one bank break (bank-level
                has_written clear on start)."""
                b, qh, h = PASSES[p]
                if qtt == 0:
                    av_state[p] = ps.tile(
                        [128, NQT, 64], F32, tag="avp", bufs=1, name=f"av{p}"
                    )
                avp = av_state[p]
                for kt in range(NKT):
                    nc.tensor.matmul(
                        avp[:, qtt, :],
                        strips[p * 16 + kt][:, qtt * 128 : (qtt + 1) * 128],
                        vht[:, b, kt, h * 64 : h * 64 + 64],
                        start=(kt == 0),
                        stop=(kt == NKT - 1),
                    )
                for kt in range(NKT):
                    nc.tensor.matmul(
                        smallp[:, qtt : qtt + 1],
                        strips[p * 16 + kt][:, qtt * 128 : (qtt + 1) * 128],
                        onesc[:],
                        start=(kt == 0),
                        stop=(kt == NKT - 1),
                    )
                if qtt == NQT - 1:
                    for kt in range(NKT):
                        strips.pop(p * 16 + kt)
                    drain(p)

            def drain(p):
                """recip of the row-sums + evacuate ctx into cc (SBUF fp16)."""
                b, qh, h = PASSES[p]
                avp = av_state.pop(p)
                if h == 0:
                    cc_tiles[(b, qh)] = cp.tile(
                        [128, NQT, 128], FP16, tag="cc", bufs=4, name=f"cc{b}{qh}"
                    )
                    rcp_tiles[(b, qh)] = cp.tile(
                        [128, 2, NQT], F32, tag="rcp", bufs=4, name=f"rcp{b}{qh}"
                    )
                cc = cc_tiles[(b, qh)]
                rcp = rcp_tiles[(b, qh)]
                with nc.allow_low_precision(reason="softmax denominator"):
                    nc.vector.reciprocal(rcp[:, h, :], smallp[:, 0:8])
                nc.vector.tensor_copy(cc[:, :, h * 64 : h * 64 + 64], avp[:])

            def outproj_a(b, qh, qtt):
                """diag(1/D) builds + transpose-normalize matmuls + ctxT copy."""
                cc = cc_tiles[(b, qh)]
                rcp = rcp_tiles[(b, qh)]
                pp = ptr_idx[0] % 2
                ptr_idx[0] += 1
                ptr = smallp[:, 256 + pp * 128 : 384 + pp * 128]
                for h in range(2):
                    dg = sp.tile(
                        [128, 128], FP16, tag="diag", bufs=3, name=f"dg{b}{qh}{qtt}{h}"
                    )
                    nc.gpsimd.tensor_scalar_mul(
                        dg[:], identf[:], rcp[:, h, qtt : qtt + 1]
                    )
                    nc.tensor.matmul(
                        ptr[h * 64 : (h + 1) * 64, :],
                        cc[:, qtt, h * 64 : (h + 1) * 64],
                        dg[:],
                        start=True,
                        stop=True,
                    )
                ctxT = sp.tile(
                    [128, 128], FP16, tag="ctxT", bufs=2, name=f"cx{b}{qh}{qtt}"
                )
                nc.vector.tensor_copy(ctxT[:], ptr[:])
                return ctxT

            def outproj_b(b, qh, qtt, ctxT, eh, tail=False):
                po = ps.tile(
                    [128, 512], F32, tag="flex", bufs=2, name=f"po{b}{qh}{qtt}{eh}"
                )
                nc.tensor.matmul(
                    po[:],
                    ctxT[:],
                    wo_sb[:, eh * 512 : (eh + 1) * 512],
                    start=True,
                    stop=True,
                )
                ob = sp.tile(
                    [128, 512], FP16, tag="ob", bufs=3, name=f"ob{b}{qh}{qtt}{eh}"
                )
                if tail and eh == 1:
                    nc.scalar.copy(ob[:], po[:])  # ACT is free in the tail
                else:
                    nc.vector.tensor_copy(ob[:], po[:])
                r0 = b * S + qh * 1024 + qtt * 128
                nc.sync.dma_start(
                    out_d[r0 : r0 + 128, eh * 512 : (eh + 1) * 512], ob[:]
                )

            # ---- filler schedule ----
            # Each filler is (ready_slot, deadline_slot, fn). Deadlines are
            # asserted; ready respects data deps (proj blocks are sequential
            # through the single pt psum tile by construction of the list).
            fillers = []

            def F(ready, deadline, fn):
                fillers.append([ready, deadline, fn])

            # KT b0 remaining blocks (blk0 in prologue): scores kt needs KT
            # block kt//4 at slot kt.
            for blk, dl in ((1, 4), (2, 8), (3, 12)):
                for hh in range(2):
                    F(0, dl - 1, lambda blk=blk, hh=hh: proj_half("K", blk, hh))
            # V b0: vh[st] needed by AV kt=st at slot st+AV_LAG (st0,1 in prologue)
            for st in range(2, 16):
                F(0, st + AV_LAG - 1, lambda st=st: v_tile(0, st))
            # QT b0 blk2,3 (qh1 -> slot 32)
            for blk in (2, 3):
                for hh in range(2):
                    F(4, 31, lambda blk=blk, hh=hh: proj_half("Q", blk, hh))
            # b1 projections (scores from slot 64; KT block (kt//4)+4 at slot
            # 64+kt; QT blk4,5 by 64, blk6,7 by 96)
            for blk, dl in ((4, 63), (5, 67), (6, 71), (7, 75)):
                for hh in range(2):
                    F(16, dl, lambda blk=blk, hh=hh: proj_half("K", blk, hh))
            for blk, dl in ((4, 63), (5, 63), (6, 94), (7, 94)):
                for hh in range(2):
                    F(
                        18 if blk < 6 else 76,
                        dl,
                        lambda blk=blk, hh=hh: proj_half("Q", blk, hh),
                    )
            # V b1: needed from slot 64+st+AV_LAG
            for st in range(16):
                F(44, 64 + st + AV_LAG - 1, lambda st=st: v_tile(1, st))

            # out-projections become ready once both h-passes of (b, qh) have
            # drained: pass p=(b,qh,1) drains at slot p*16+15+AV_LAG.
            ctxT_holder = {}

            def op_a(b, qh, qtt):
                ctxT_holder[(b, qh, qtt)] = outproj_a(b, qh, qtt)

            def op_b(b, qh, qtt, eh):
                outproj_b(b, qh, qtt, ctxT_holder[(b, qh, qtt)], eh)

            # deferred AV: pass p's per-q-tile accumulations run during
            # pass p+1 (they need all 16 strips of pass p)
            for p in range(len(PASSES) - 1):
                for qtt in range(NQT):
                    F(
                        (p + 1) * 16 + qtt,
                        (p + 1) * 16 + qtt + 6,
                        lambda p=p, qtt=qtt: av_qt(p, qtt),
                    )

            for gi, (b, qh) in enumerate([(0, 0), (0, 1), (1, 0)]):
                rdy = max((PASSES.index((b, qh, 1)) + 1) * 16 + 9, 64)
                for qtt in range(NQT):
                    F(rdy + 2 * qtt, NSLOT - 1, lambda b=b, qh=qh, qtt=qtt: op_a(b, qh, qtt))
                    F(rdy + 2 * qtt, NSLOT - 1, lambda b=b, qh=qh, qtt=qtt: op_b(b, qh, qtt, 0))
                    F(rdy + 2 * qtt + 1, NSLOT - 1, lambda b=b, qh=qh, qtt=qtt: op_b(b, qh, qtt, 1))

            fillers.sort(key=lambda f: (f[0], f[1]))

            # ---- prologue projections (needed before slot 0) ----
            for hh in range(2):
                proj_half("Q", 0, hh)
            for hh in range(2):
                proj_half("K", 0, hh)
            for hh in range(2):
                proj_half("Q", 1, hh)
            v_tile(0, 0)
            v_tile(0, 1)

            # ---- main pipeline ----
            done = [False] * len(fillers)

            def pop_fillers(s, budget):
                """Emit due/ready fillers for slot s up to a PE-cost budget."""
                spent = 0
                # urgent first (deadline within 2 slots), then earliest-ready
                for idx, (rdy, dl, fn) in enumerate(fillers):
                    if done[idx] or rdy > s:
                        continue
                    urgent = dl <= s + 2
                    if not urgent and spent >= budget:
                        continue
                    fn()
                    done[idx] = True
                    spent += 1
                    if spent >= budget and not any(
                        (not done[j]) and fillers[j][0] <= s and fillers[j][1] <= s + 2
                        for j in range(len(fillers))
                    ):
                        break
                return spent

            for s in range(NSLOT):
                p, kt = divmod(s, NKT)
                b, qh, h = PASSES[p]
                strips[s] = scores_exp(b, qh, h, kt, s)
                pop_fillers(s, budget=1)

            # ---- epilogue: per-q-tile pipeline so PE/DVE/ACT overlap ----
            for idx, (rdy, dl, fn) in enumerate(fillers):
                if not done[idx]:
                    fn()
                    done[idx] = True
            p7 = len(PASSES) - 1
            b7, qh7, h7 = PASSES[p7]
            cc7 = cc_tiles[(b7, qh7)]
            rcp7 = rcp_tiles[(b7, qh7)]
            avp7 = ps.tile([128, NQT, 64], F32, tag="avp", bufs=1, name="av7")
            av_state[p7] = avp7
            for qtt in range(NQT):
                # AV + D for this q-tile
                for kt in range(NKT):
                    nc.tensor.matmul(
                        avp7[:, qtt, :],
                        strips[p7 * 16 + kt][:, qtt * 128 : (qtt + 1) * 128],
                        vht[:, b7, kt, h7 * 64 : h7 * 64 + 64],
                        start=(kt == 0),
                        stop=(kt == NKT - 1),
                    )
                for kt in range(NKT):
                    nc.tensor.matmul(
                        smallp[:, qtt : qtt + 1],
                        strips[p7 * 16 + kt][:, qtt * 128 : (qtt + 1) * 128],
                        onesc[:],
                        start=(kt == 0),
                        stop=(kt == NKT - 1),
                    )
                with nc.allow_low_precision(reason="softmax denominator"):
                    nc.vector.reciprocal(
                        rcp7[:, h7, qtt : qtt + 1], smallp[:, qtt : qtt + 1]
                    )
                nc.vector.tensor_copy(
                    cc7[:, qtt, h7 * 64 : h7 * 64 + 64], avp7[:, qtt, :]
                )
                # normalize-transpose + out-projection, po in the freed pscr ring
                pp = ptr_idx[0] % 2
                ptr_idx[0] += 1
                ptr = smallp[:, 256 + pp * 128 : 384 + pp * 128]
                for h in range(2):
                    dg = sp.tile(
                        [128, 128], FP16, tag="diag", bufs=3, name=f"edg{qtt}{h}"
                    )
                    nc.gpsimd.tensor_scalar_mul(
                        dg[:], identf[:], rcp7[:, h, qtt : qtt + 1]
                    )
                    nc.tensor.matmul(
                        ptr[h * 64 : (h + 1) * 64, :],
                        cc7[:, qtt, h * 64 : (h + 1) * 64],
                        dg[:],
                        start=True,
                        stop=True,
                    )
                ctxT = sp.tile([128, 128], FP16, tag="ctxT", bufs=2, name=f"ecx{qtt}")
                nc.vector.tensor_copy(ctxT[:], ptr[:])
                po = ps.tile([128, 1024], F32, tag="pscr", bufs=2, name=f"epo{qtt}")
                for eh in range(2):
                    nc.tensor.matmul(
                        po[:, eh * 512 : (eh + 1) * 512],
                        ctxT[:],
                        wo_sb[:, eh * 512 : (eh + 1) * 512],
                        start=True,
                        stop=True,
                    )
                ob = sp.tile([128, 1024], FP16, tag="eob", bufs=3, name=f"eob{qtt}")
                nc.vector.tensor_copy(ob[:, 0:512], po[:, 0:512])
                nc.scalar.copy(ob[:, 512:1024], po[:, 512:1024])
                r0 = b7 * S + qh7 * 1024 + qtt * 128
                nc.sync.dma_start(out_d[r0 : r0 + 128, :], ob[:])
            for kt in range(NKT):
                strips.pop(p7 * 16 + kt)

    nc.compile()
    return nc


def _get_nc():
    global _CACHED_NC
    if _CACHED_NC is None:
        _CACHED_NC = _build()
    return _CACHED_NC


def _in_maps(q, Wq, Wk, Wv, Wo):
    q32 = np.asarray(q, np.float32).reshape(BS, NCH, 128)
    qt = np.ascontiguousarray(q32.transpose(2, 1, 0)).astype(ml_dtypes.bfloat16)

    def warr(W, sl):
        w = np.asarray(W, np.float32)[:, sl].reshape(NCH, 128, 128).transpose(1, 0, 2)
        return np.ascontiguousarray(w).astype(ml_dtypes.bfloat16)

    Wo32 = np.asarray(Wo, np.float32)
    maps = []
    for c in range(NCORES):
        sl = slice(c * 128, (c + 1) * 128)
        maps.append(
            {
                "qt": qt,
                "wq": warr(Wq, sl),
                "wk": warr(Wk, sl),
                "wv": warr(Wv, sl),
                "wo": np.ascontiguousarray(Wo32[sl, :]).astype(np.float16),
            }
        )
    return maps


def run(q, Wq, Wk, Wv, Wo, bo, trace=False):
    nc = _get_nc()
    res = run_bass_kernel_spmd(
        nc, _in_maps(q, Wq, Wk, Wv, Wo), list(range(NCORES)), trace=trace
    )
    acc = np.zeros((BS, D), np.float32)
    for r in res.results:
        acc += r["out"].astype(np.float32)
    out = (acc + np.asarray(bo, np.float32)).astype(np.float32)
    return out.reshape(B, S, D), res


def kernel(q, Wq, Wk, Wv, Wo, bo):
    out, _ = run(q, Wq, Wk, Wv, Wo, bo)
    return out


# revision 12
# speedup vs baseline: 1.0608x; 1.0071x over previous
"""Multi-head attention (B=2, S=2048, D=1024, H=16) on 8 TRN2 NeuronCores.

Tensor-parallel over heads: core c owns heads {2c, 2c+1} (a 128-wide slice of
the QKV projections / a 128-row slice of Wo). Each core computes its partial
out-projection in fp16; the host sums the 8 partials and adds the bias.

v2 layout (all-bf16/fp16 matmuls, q-major AV):
  - QT/KT = (q @ Wq|k)^T in [dh2h, bs] layout (lhsT = W chunk, rhs = qT chunk)
  - V projected directly into vh [s(=k), dv2h] tiles (lhsT = qT chunk)
  - scores k-major: sT[k, q] = KT_h-slice.T @ QT_h-slice, psum [128, 1024]
  - exp on ScalarE (fused 1/8 scale, no max subtraction; scores ~N(0,1)),
    strip bf16 in SBUF
  - AV q-major: ctx[q, dv] += strip_slice.T @ vh_slice accumulated over the
    16 k-tiles; row-sums via an extra ones-column matmul per q-tile (out
    free size 1 -> ~free)
  - normalization fused into the ctx transpose: a regular matmul against
    diag(1/D) (built by GPSIMD from an fp16 identity) yields
    ctxT[dvh, q] = ctx[q, dvh]/D_q
  - out projection: po[q, e] = ctxT.T-slice @ Wo-slice, fp16 partial out

Scheduling: a global 128-slot pipeline (slot = one (pass, k-tile)); each slot
carries scores+exp+AV(lag 3) plus "filler" PE work (projections, V, out-proj)
budgeted so the tensor engine never idles (the cost model's p-state ramp
penalizes every PE idle gap). Warmup dummy matmuls cover the initial DMA wait.
"""

import numpy as np
import ml_dtypes

import concourse.bass as bass
import concourse.mybir as mybir
import concourse.tile as tile
from concourse import bacc
from concourse.bass_utils import run_bass_kernel_spmd
from concourse.masks import make_identity

BF16 = mybir.dt.bfloat16
FP16 = mybir.dt.float16
F32 = mybir.dt.float32
EXP = mybir.ActivationFunctionType.Exp

B, S, D = 2, 2048, 1024
H, DH = 16, 64
NCORES = 8
BS = B * S  # 4096
NCH = D // 128  # 8 contraction chunks for the projections
NKT = S // 128  # 16 k-tiles per batch
NQT = 8  # q-tiles (128) per q-half
AV_LAG = 3  # slots between exp(s) and AV(s)
N_DUMMY = 10  # warmup matmuls riding out the DMA wait + p-state ramp

# pass p = (b, qh, h); slot s = p*16 + kt
PASSES = [(b, qh, h) for b in range(B) for qh in range(2) for h in range(2)]
NSLOT = len(PASSES) * NKT  # 128

_CACHED_NC = None


def _build():
    nc = bacc.Bacc("TRN2", target_bir_lowering=False, debug=False)

    qt_d = nc.dram_tensor("qt", [128, NCH, BS], BF16, kind="ExternalInput")
    wq_d = nc.dram_tensor("wq", [128, NCH, 128], BF16, kind="ExternalInput")
    wk_d = nc.dram_tensor("wk", [128, NCH, 128], BF16, kind="ExternalInput")
    wv_d = nc.dram_tensor("wv", [128, NCH, 128], BF16, kind="ExternalInput")
    wo_d = nc.dram_tensor("wo", [128, D], FP16, kind="ExternalInput")
    out_d = nc.dram_tensor("out", [BS, D], FP16, kind="ExternalOutput")

    with tile.TileContext(nc) as tc:
        with (
            tc.tile_pool(name="cp", bufs=1) as cp,
            tc.tile_pool(name="sp", bufs=1) as sp,
            tc.tile_pool(name="ps", bufs=1, space="PSUM") as ps,
        ):
            # ---- persistent SBUF ----
            qt_sb = cp.tile([128, NCH, BS], BF16, tag="qt")
            wq_sb = cp.tile([128, NCH, 128], BF16, tag="wq")
            wk_sb = cp.tile([128, NCH, 128], BF16, tag="wk")
            wv_sb = cp.tile([128, NCH, 128], BF16, tag="wv")
            wo_sb = cp.tile([128, D], FP16, tag="wo")
            QTt = cp.tile([128, BS], BF16, tag="QT")  # [2h*dh, b*s]
            KTt = cp.tile([128, BS], BF16, tag="KT")
            vht = cp.tile([128, B, NKT, 128], BF16, tag="vh")  # [k, b, kt, dv2h]
            onesc = cp.tile([128, 1], BF16, tag="ones")
            identf = cp.tile([128, 128], FP16, tag="ident")
            dmy_w = cp.tile([128, 128], BF16, tag="dmy_w")
            dmy_a = cp.tile([128, 384], BF16, tag="dmy_a")

            # one shared PSUM bank for the small tiles (bank-granular
            # allocator): per-kt D partials [*,0:128] as [8qt,16kt], vv
            # [*,128:256], ptr ping-pong [*,256:384] / [*,384:512]
            smallp = ps.tile([128, 512], F32, tag="small", bufs=1)

            nc.vector.memset(onesc[:], 1.0)
            nc.vector.memset(dmy_w[:], 0.0)
            nc.vector.memset(dmy_a[:], 0.0)
            make_identity(nc, identf[:])

            # ---- warmup dummies (PE busy during DMA wait; ride the ramp) ----
            for i in range(N_DUMMY):
                pd = ps.tile(
                    [128, 1024], F32, tag="pscr", bufs=2, name=f"dmy{i}"
                )
                nc.tensor.matmul(
                    pd[:, 0:384], dmy_w[:], dmy_a[:], start=True, stop=True
                )

            # ---- input DMAs: critical wave first, issue spread across
            # the SP/ACT/DVE DGE queues (issue itself serializes per engine)
            nc.sync.dma_start(wq_sb[:], wq_d.ap())
            nc.scalar.dma_start(wk_sb[:], wk_d.ap())
            nc.sync.dma_start(qt_sb[:, 0:4, 0:512], qt_d[:, 0:4, 0:512])
            nc.scalar.dma_start(qt_sb[:, 4:8, 0:512], qt_d[:, 4:8, 0:512])
            nc.sync.dma_start(qt_sb[:, 0:4, 512:1024], qt_d[:, 0:4, 512:1024])
            nc.scalar.dma_start(qt_sb[:, 4:8, 512:1024], qt_d[:, 4:8, 512:1024])
            nc.sync.dma_start(wv_sb[:], wv_d.ap())
            nc.scalar.dma_start(qt_sb[:, 4:8, 1024:2048], qt_d[:, 4:8, 1024:2048])
            nc.sync.dma_start(qt_sb[:, 0:4, 1024:2048], qt_d[:, 0:4, 1024:2048])
            nc.scalar.dma_start(qt_sb[:, 4:8, 2048:3072], qt_d[:, 4:8, 2048:3072])
            nc.sync.dma_start(qt_sb[:, 0:4, 2048:3072], qt_d[:, 0:4, 2048:3072])
            nc.scalar.dma_start(qt_sb[:, 4:8, 3072:BS], qt_d[:, 4:8, 3072:BS])
            nc.sync.dma_start(qt_sb[:, 0:4, 3072:BS], qt_d[:, 0:4, 3072:BS])
            nc.scalar.dma_start(wo_sb[:], wo_d[:, :])

            # ---- helpers ----
            def proj_half(which, blk, half):
                """4 of the 8 contraction-chunk matmuls of one 512-col
                projection block. Each half owns its flex psum tile for its
                whole lifecycle (alloc->mms->evict) so the shared flex ring
                can never deadlock the PE FIFO. half 1 adds into the dst."""
                w_sb, dst = (wq_sb, QTt) if which == "Q" else (wk_sb, KTt)
                s0 = blk * 512
                pt = ps.tile(
                    [128, 512], F32, tag="flex", bufs=2, name=f"pt_{which}{blk}_{half}"
                )
                for c in range(half * 4, half * 4 + 4):
                    nc.tensor.matmul(
                        pt[:],
                        w_sb[:, c, :],
                        qt_sb[:, c, s0 : s0 + 512],
                        start=(c == half * 4),
                        stop=(c == half * 4 + 3),
                    )
                if half == 0:
                    nc.vector.tensor_copy(dst[:, s0 : s0 + 512], pt[:])
                else:
                    nc.vector.tensor_tensor(
                        dst[:, s0 : s0 + 512],
                        dst[:, s0 : s0 + 512],
                        pt[:],
                        mybir.AluOpType.add,
                    )

            def v_tile(bb, st):
                """Project one 128-row s-tile of batch bb into vh."""
                vv = smallp[:, 128:256]
                s0 = bb * S + st * 128
                for c in range(NCH):
                    nc.tensor.matmul(
                        vv[:],
                        qt_sb[:, c, s0 : s0 + 128],
                        wv_sb[:, c, :],
                        start=(c == 0),
                        stop=(c == 7),
                    )
                nc.vector.tensor_copy(vht[:, bb, st, :], vv[:])

            cc_tiles = {}
            rcp_tiles = {}
            diag_tiles = {}
            ptr_idx = [0]

            def scores_exp(b, qh, h, kt, s):
                hp = h * 64
                k0 = b * S + kt * 128
                q0 = b * S + qh * 1024
                pscr = ps.tile([128, 1024], F32, tag="pscr", bufs=2, name=f"sc{s}")
                for j in range(2):
                    nc.tensor.matmul(
                        pscr[:, j * 512 : (j + 1) * 512],
                        KTt[hp : hp + 64, k0 : k0 + 128],
                        QTt[hp : hp + 64, q0 + j * 512 : q0 + (j + 1) * 512],
                        start=True,
                        stop=True,
                    )
                strip = sp.tile(
                    [128, 1024], BF16, tag="strip", bufs=32, name=f"st{s}"
                )
                nc.scalar.activation(strip[:], pscr[:], EXP, scale=0.125)
                return strip

            av_state = {}
            strips = {}

            def av_qt(p, qtt):
                """Full 16-kt accumulation for one q-tile of pass p (runs as
                a filler during pass p+1). Groups are contiguous: interleaved
                PSUM accumulation groups in one bank break (bank-level
                has_written clear on start)."""
                b, qh, h = PASSES[p]
                if qtt == 0:
                    av_state[p] = ps.tile(
                        [128, NQT, 64], F32, tag="avp", bufs=1, name=f"av{p}"
                    )
                avp = av_state[p]
                for kt in range(NKT):
                    nc.tensor.matmul(
                        avp[:, qtt, :],
                        strips[p * 16 + kt][:, qtt * 128 : (qtt + 1) * 128],
                        vht[:, b, kt, h * 64 : h * 64 + 64],
                        start=(kt == 0),
                        stop=(kt == NKT - 1),
                    )
                for kt in range(NKT):
                    nc.tensor.matmul(
                        smallp[:, qtt : qtt + 1],
                        strips[p * 16 + kt][:, qtt * 128 : (qtt + 1) * 128],
                        onesc[:],
                        start=(kt == 0),
                        stop=(kt == NKT - 1),
                    )
                if qtt == NQT - 1:
                    for kt in range(NKT):
                        strips.pop(p * 16 + kt)
                    drain(p)

            def drain(p):
                """recip of the row-sums + evacuate ctx into cc (SBUF fp16)."""
                b, qh, h = PASSES[p]
                avp = av_state.pop(p)
                if h == 0:
                    cc_tiles[(b, qh)] = cp.tile(
                        [128, NQT, 128], FP16, tag="cc", bufs=4, name=f"cc{b}{qh}"
                    )
                    rcp_tiles[(b, qh)] = cp.tile(
                        [128, 2, NQT], F32, tag="rcp", bufs=4, name=f"rcp{b}{qh}"
                    )
                cc = cc_tiles[(b, qh)]
                rcp = rcp_tiles[(b, qh)]
                with nc.allow_low_precision(reason="softmax denominator"):
                    nc.vector.reciprocal(rcp[:, h, :], smallp[:, 0:8])
                nc.vector.tensor_copy(cc[:, :, h * 64 : h * 64 + 64], avp[:])
                if h == 1:
                    # pre-build all diag(1/D) tiles on GPSIMD so the
                    # out-projection transposes never wait on Pool
                    dgs = []
                    for qtt in range(NQT):
                        for hh in range(2):
                            dg = sp.tile(
                                [128, 128], FP16, tag="diag", bufs=36,
                                name=f"dg{b}{qh}{qtt}{hh}",
                            )
                            nc.gpsimd.tensor_scalar_mul(
                                dg[:], identf[:], rcp[:, hh, qtt : qtt + 1]
                            )
                            dgs.append(dg)
                    diag_tiles[(b, qh)] = dgs

            def outproj_a(b, qh, qtt):
                """transpose-normalize matmuls (diag pre-built) + ctxT copy."""
                cc = cc_tiles[(b, qh)]
                pp = ptr_idx[0] % 2
                ptr_idx[0] += 1
                ptr = smallp[:, 256 + pp * 128 : 384 + pp * 128]
                for h in range(2):
                    dg = diag_tiles[(b, qh)][qtt * 2 + h]
                    nc.tensor.matmul(
                        ptr[h * 64 : (h + 1) * 64, :],
                        cc[:, qtt, h * 64 : (h + 1) * 64],
                        dg[:],
                        start=True,
                        stop=True,
                    )
                ctxT = sp.tile(
                    [128, 128], FP16, tag="ctxT", bufs=4, name=f"cx{b}{qh}{qtt}"
                )
                nc.vector.tensor_copy(ctxT[:], ptr[:])
                return ctxT

            def outproj_b(b, qh, qtt, ctxT, eh, tail=False):
                po = ps.tile(
                    [128, 512], F32, tag="flex", bufs=2, name=f"po{b}{qh}{qtt}{eh}"
                )
                nc.tensor.matmul(
                    po[:],
                    ctxT[:],
                    wo_sb[:, eh * 512 : (eh + 1) * 512],
                    start=True,
                    stop=True,
                )
                ob = sp.tile(
                    [128, 512], FP16, tag="ob", bufs=3, name=f"ob{b}{qh}{qtt}{eh}"
                )
                if tail and eh == 1:
                    nc.scalar.copy(ob[:], po[:])  # ACT is free in the tail
                else:
                    nc.vector.tensor_copy(ob[:], po[:])
                r0 = b * S + qh * 1024 + qtt * 128
                nc.sync.dma_start(
                    out_d[r0 : r0 + 128, eh * 512 : (eh + 1) * 512], ob[:]
                )

            # ---- filler schedule ----
            # Each filler is (ready_slot, deadline_slot, fn). Deadlines are
            # asserted; ready respects data deps (proj blocks are sequential
            # through the single pt psum tile by construction of the list).
            fillers = []

            def F(ready, deadline, fn):
                fillers.append([ready, deadline, fn])

            # KT b0 remaining blocks (blk0 in prologue): scores kt needs KT
            # block kt//4 at slot kt.
            for blk, dl in ((1, 4), (2, 8), (3, 12)):
                for hh in range(2):
                    F(0, dl - 1, lambda blk=blk, hh=hh: proj_half("K", blk, hh))
            # V b0: vh[st] needed by AV kt=st at slot st+AV_LAG (st0,1 in prologue)
            for st in range(2, 16):
                F(0, st + AV_LAG - 1, lambda st=st: v_tile(0, st))
            # QT b0 blk2,3 (qh1 -> slot 32)
            for blk in (2, 3):
                for hh in range(2):
                    F(4, 31, lambda blk=blk, hh=hh: proj_half("Q", blk, hh))
            # b1 projections (scores from slot 64; KT block (kt//4)+4 at slot
            # 64+kt; QT blk4,5 by 64, blk6,7 by 96)
            for blk, dl in ((4, 63), (5, 67), (6, 71), (7, 75)):
                for hh in range(2):
                    F(16, dl, lambda blk=blk, hh=hh: proj_half("K", blk, hh))
            for blk, dl in ((4, 63), (5, 63), (6, 94), (7, 94)):
                for hh in range(2):
                    F(
                        18 if blk < 6 else 76,
                        dl,
                        lambda blk=blk, hh=hh: proj_half("Q", blk, hh),
                    )
            # V b1: needed from slot 64+st+AV_LAG
            for st in range(16):
                F(44, 64 + st + AV_LAG - 1, lambda st=st: v_tile(1, st))

            # out-projections become ready once both h-passes of (b, qh) have
            # drained: pass p=(b,qh,1) drains at slot p*16+15+AV_LAG.
            ctxT_holder = {}

            def op_a(b, qh, qtt):
                ctxT_holder[(b, qh, qtt)] = outproj_a(b, qh, qtt)

            def op_b(b, qh, qtt, eh):
                outproj_b(b, qh, qtt, ctxT_holder[(b, qh, qtt)], eh)

            # deferred AV: pass p's per-q-tile accumulations run during
            # pass p+1 (they need all 16 strips of pass p)
            for p in range(len(PASSES) - 1):
                for qtt in range(NQT):
                    F(
                        (p + 1) * 16 + qtt,
                        (p + 1) * 16 + qtt + 6,
                        lambda p=p, qtt=qtt: av_qt(p, qtt),
                    )

            for gi, (b, qh) in enumerate([(0, 0), (0, 1), (1, 0)]):
                rdy = max((PASSES.index((b, qh, 1)) + 1) * 16 + 9, 64)
                for qtt in range(NQT):
                    F(rdy + 2 * qtt, NSLOT - 1, lambda b=b, qh=qh, qtt=qtt: op_a(b, qh, qtt))
                for qtt in range(NQT):
                    F(rdy + 2 * qtt + 2, NSLOT - 1, lambda b=b, qh=qh, qtt=qtt: op_b(b, qh, qtt, 0))
                    F(rdy + 2 * qtt + 3, NSLOT - 1, lambda b=b, qh=qh, qtt=qtt: op_b(b, qh, qtt, 1))

            fillers.sort(key=lambda f: (f[0], f[1]))

            # ---- prologue projections (needed before slot 0) ----
            for hh in range(2):
                proj_half("Q", 0, hh)
            for hh in range(2):
                proj_half("K", 0, hh)
            for hh in range(2):
                proj_half("Q", 1, hh)
            v_tile(0, 0)
            v_tile(0, 1)

            # ---- main pipeline ----
            done = [False] * len(fillers)

            def pop_fillers(s, budget):
                """Emit due/ready fillers for slot s up to a PE-cost budget."""
                spent = 0
                # urgent first (deadline within 2 slots), then earliest-ready
                for idx, (rdy, dl, fn) in enumerate(fillers):
                    if done[idx] or rdy > s:
                        continue
                    urgent = dl <= s + 2
                    if not urgent and spent >= budget:
                        continue
                    fn()
                    done[idx] = True
                    spent += 1
                    if spent >= budget and not any(
                        (not done[j]) and fillers[j][0] <= s and fillers[j][1] <= s + 2
                        for j in range(len(fillers))
                    ):
                        break
                return spent

            for s in range(NSLOT):
                p, kt = divmod(s, NKT)
                b, qh, h = PASSES[p]
                strips[s] = scores_exp(b, qh, h, kt, s)
                pop_fillers(s, budget=1)

            # ---- epilogue: phase 1 (AV/D + recip + cc + diags), then
            # phase 2 (transpose-normalize + out-proj), so PE never waits on
            # the DVE/Pool round-trips
            for idx, (rdy, dl, fn) in enumerate(fillers):
                if not done[idx]:
                    fn()
                    done[idx] = True
            p7 = len(PASSES) - 1
            b7, qh7, h7 = PASSES[p7]
            cc7 = cc_tiles[(b7, qh7)]
            rcp7 = rcp_tiles[(b7, qh7)]
            avp7 = ps.tile([128, NQT, 64], F32, tag="avp", bufs=1, name="av7")
            dgs7 = []
            for qtt in range(NQT):
                for kt in range(NKT):
                    nc.tensor.matmul(
                        avp7[:, qtt, :],
                        strips[p7 * 16 + kt][:, qtt * 128 : (qtt + 1) * 128],
                        vht[:, b7, kt, h7 * 64 : h7 * 64 + 64],
                        start=(kt == 0),
                        stop=(kt == NKT - 1),
                    )
                for kt in range(NKT):
                    nc.tensor.matmul(
                        smallp[:, qtt : qtt + 1],
                        strips[p7 * 16 + kt][:, qtt * 128 : (qtt + 1) * 128],
                        onesc[:],
                        start=(kt == 0),
                        stop=(kt == NKT - 1),
                    )
                with nc.allow_low_precision(reason="softmax denominator"):
                    nc.vector.reciprocal(
                        rcp7[:, h7, qtt : qtt + 1], smallp[:, qtt : qtt + 1]
                    )
                nc.vector.tensor_copy(
                    cc7[:, qtt, h7 * 64 : h7 * 64 + 64], avp7[:, qtt, :]
                )
                for hh in range(2):
                    dg = sp.tile(
                        [128, 128], FP16, tag="diag", bufs=36, name=f"edg{qtt}{hh}"
                    )
                    nc.gpsimd.tensor_scalar_mul(
                        dg[:], identf[:], rcp7[:, hh, qtt : qtt + 1]
                    )
                    dgs7.append(dg)
            for qtt in range(NQT):
                pp = ptr_idx[0] % 2
                ptr_idx[0] += 1
                ptr = smallp[:, 256 + pp * 128 : 384 + pp * 128]
                for h in range(2):
                    nc.tensor.matmul(
                        ptr[h * 64 : (h + 1) * 64, :],
                        cc7[:, qtt, h * 64 : (h + 1) * 64],
                        dgs7[qtt * 2 + h][:],
                        start=True,
                        stop=True,
                    )
                ctxT = sp.tile([128, 128], FP16, tag="ctxT", bufs=4, name=f"ecx{qtt}")
                nc.vector.tensor_copy(ctxT[:], ptr[:])
                po = ps.tile([128, 1024], F32, tag="pscr", bufs=2, name=f"epo{qtt}")
                for eh in range(2):
                    nc.tensor.matmul(
                        po[:, eh * 512 : (eh + 1) * 512],
                        ctxT[:],
                        wo_sb[:, eh * 512 : (eh + 1) * 512],
                        start=True,
                        stop=True,
                    )
                ob = sp.tile([128, 1024], FP16, tag="eob", bufs=3, name=f"eob{qtt}")
                nc.vector.tensor_copy(ob[:, 0:512], po[:, 0:512])
                nc.scalar.copy(ob[:, 512:1024], po[:, 512:1024])
                r0 = b7 * S + qh7 * 1024 + qtt * 128
                nc.sync.dma_start(out_d[r0 : r0 + 128, :], ob[:])
            for kt in range(NKT):
                strips.pop(p7 * 16 + kt)

    nc.compile()
    return nc


def _get_nc():
    global _CACHED_NC
    if _CACHED_NC is None:
        _CACHED_NC = _build()
    return _CACHED_NC


def _in_maps(q, Wq, Wk, Wv, Wo):
    q32 = np.asarray(q, np.float32).reshape(BS, NCH, 128)
    qt = np.ascontiguousarray(q32.transpose(2, 1, 0)).astype(ml_dtypes.bfloat16)

    def warr(W, sl):
        w = np.asarray(W, np.float32)[:, sl].reshape(NCH, 128, 128).transpose(1, 0, 2)
        return np.ascontiguousarray(w).astype(ml_dtypes.bfloat16)

    Wo32 = np.asarray(Wo, np.float32)
    maps = []
    for c in range(NCORES):
        sl = slice(c * 128, (c + 1) * 128)
        maps.append(
            {
                "qt": qt,
                "wq": warr(Wq, sl),
                "wk": warr(Wk, sl),
                "wv": warr(Wv, sl),
                "wo": np.ascontiguousarray(Wo32[sl, :]).astype(np.float16),
            }
        )
    return maps


def run(q, Wq, Wk, Wv, Wo, bo, trace=False):
    nc = _get_nc()
    res = run_bass_kernel_spmd(
        nc, _in_maps(q, Wq, Wk, Wv, Wo), list(range(NCORES)), trace=trace
    )
    acc = np.zeros((BS, D), np.float32)
    for r in res.results:
        acc += r["out"].astype(np.float32)
    out = (acc + np.asarray(bo, np.float32)).astype(np.float32)
    return out.reshape(B, S, D), res


def kernel(q, Wq, Wk, Wv, Wo, bo):
    out, _ = run(q, Wq, Wk, Wv, Wo, bo)
    return out
